# revision 2
# baseline (speedup 1.0000x reference)
"""Trainium2 Bass kernel for batch-all triplet margin loss (N=512, D=128).

Math:
  dist[i,g] = ||x_i - x_g||  (the reference's +1e-6 inside the norm shifts
  d2 by ~3e-5 -- far below bf16 noise, so it is dropped)
  loss = mean over valid (i,j,g) of relu(dist[i,j] - dist[i,g] + margin)
  valid: labels[j]==labels[i], j != i, labels[g] != labels[i]

Device strategy (SPMD over 8 cores, 64 anchors each, rows duplicated x2 so
each main-loop pass covers two positive ordinals):
  - d2 blocks build in PSUM as (-2 X_a X^T) + a rank-20 augment matmul
    carrying r_g (hi/lo bf16 split), a 2^20 same-class mask (rank-16), and
    r_i (hi/lo rows with ones-rhs) -- so the sqrt needs NO bias and does
    not wait on the bias table.  Masked distances come out EXACTLY 1024.
  - The fp32 bias columns (pb) ride inside the pa DMA as bitcast bf16
    columns: one less DMA on the critical path.
  - main loop: 16 ordinal passes (ranks 0..31), plus a rowsum column
    (DVE max-pass with bias -1e9) to recover relu sums from the DVE
    max-accumulate trick; 2 ordinals go to ACT (relu w/ accum), 14 to DVE.
  - overflow (ranks >= 32 for every class with >32 members) goes to a
    per-core custom tail block: 128 (anchor, rank) slots, its own
    product+augment and sqrt, one ACT relu pass.  886 pairs fit in
    8*128 = 1024 tail slots.
  - acc [128, 18]: cols 0..15 ordinals | 16 tail | 17 rowsum.
"""

import numpy as np
import ml_dtypes

BF = ml_dtypes.bfloat16
N, D, C = 512, 128, 16
NCORES = 8
APC = N // NCORES  # 64 anchors per core
MASK = float(2.0**20)
PADBIAS = 1.0e9

UMAIN = 16                 # main ordinal-pair passes (ranks 0..2*UMAIN-1)
DVE_US = tuple(range(14))  # ordinals on DVE (max/add + rowsum correction)
ACT_US = (14, 15)          # ordinals on ACT (relu direct)
PBW = 18                   # fp32 bias cols: 16 ordinals | tail | rowsum
PAW = 640 + 2 * PBW        # pa cols: xia(128) | xga(512) | pb bits(36)

_CACHE = {}


def _build_program(margin):
    import concourse.bacc as bacc
    import concourse.tile as tile
    from concourse import mybir

    fp32 = mybir.dt.float32
    bf16 = mybir.dt.bfloat16
    AF = mybir.ActivationFunctionType
    OP = mybir.AluOpType

    nc = bacc.Bacc("TRN2", target_bir_lowering=False, debug=False)
    pa = nc.declare_dram_parameter("pa", [128, PAW], bf16, isOutput=False)
    pk2 = nc.declare_dram_parameter("pk2", [20, 768], bf16, isOutput=False)
    pa47 = nc.declare_dram_parameter("pa47", [128, 128], bf16, isOutput=False)
    acc_out = nc.declare_dram_parameter("acc", [128, PBW], fp32, isOutput=True)

    with tile.TileContext(nc) as tc:
        with (
            tc.tile_pool(name="io", bufs=1) as io,
            tc.tile_pool(name="work", bufs=1) as work,
            tc.tile_pool(name="psum", bufs=1, space="PSUM") as psum,
        ):
            t_pa = io.tile([128, PAW], bf16)
            t_pk2 = io.tile([20, 768], bf16)
            t_pa47 = io.tile([128, 128], bf16)
            # dummy act first in program order -> LoadActFuncSet at the top
            # of the ACT stream, overlapping the input DMAs
            t_dummy = work.tile([128, 1], fp32, tag="dummy")
            nc.gpsimd.memset(t_dummy[:], 1.0)
            t_dummy2 = work.tile([128, 1], bf16, tag="dummy2")
            nc.scalar.activation(t_dummy2[:], t_dummy[:], AF.Sqrt)
            # input DMAs, program order = issue order on the SP queue
            nc.sync.dma_start(t_pa[:], pa[:])
            nc.sync.dma_start(t_pk2[:], pk2[:])
            nc.sync.dma_start(t_pa47[:], pa47[:])

            xia = t_pa[:, 0:128]            # X_Idup^T [d, 128]
            xga = t_pa[:, 128:640]          # -2 X^T [d, 512]
            pbv = t_pa[:, 640:PAW].bitcast(fp32)  # [128, PBW] fp32 biases

            # ---- main d2: [slot(128), g(512)] ----
            p_d2 = psum.tile([128, N], fp32, tag="d2")
            with tc.high_priority():
                nc.tensor.matmul(p_d2[:], xia, xga, start=True, stop=False)
                nc.tensor.matmul(
                    p_d2[:], t_pk2[0:20, 0:128], t_pk2[0:20, 128:640],
                    start=False, stop=True,
                )
            # ---- tail d2: [tail-slot(128), g(512)] ----
            p_d247 = psum.tile([128, N], fp32, tag="d247")
            nc.tensor.matmul(p_d247[:], t_pa47[:], xga, start=True, stop=False)
            nc.tensor.matmul(
                p_d247[:], t_pk2[0:20, 640:768], t_pk2[0:20, 128:640],
                start=False, stop=True,
            )

            # ---- sqrts (ACT, biasless: r_i rides in the augment) ----
            t_bneg = work.tile([128, N], bf16, tag="bneg")
            nc.scalar.activation(t_bneg[:], p_d2[:], AF.Sqrt)
            t_bneg47 = work.tile([128, N], bf16, tag="bneg47")
            nc.scalar.activation(t_bneg47[:], p_d247[:], AF.Sqrt)

            # ---- main loop ----
            t_acc = work.tile([128, PBW], fp32, tag="acc")
            t_trash_d = [
                work.tile([128, N], bf16, name=f"trd{i}", tag=f"trd{i}")
                for i in range(2)
            ]
            t_trash_a = [
                work.tile([128, N], bf16, name=f"tra{i}", tag=f"tra{i}")
                for i in range(2)
            ]
            # rowsum(d): DVE max-pass with bias -1e9
            nc.vector.tensor_scalar(
                t_trash_d[1][:], t_bneg[:], pbv[:, 17:18], None,
                op0=OP.max, op1=OP.add, accum_out=t_acc[:, 17:18],
            )
            nd = na = 0
            for u in range(UMAIN):
                if u in ACT_US:
                    nc.scalar.activation(
                        t_trash_a[na % 2][:], t_bneg[:], AF.Relu,
                        bias=pbv[:, u : u + 1], scale=-1.0,
                        accum_out=t_acc[:, u : u + 1],
                    )
                    na += 1
                else:
                    nc.vector.tensor_scalar(
                        t_trash_d[nd % 2][:], t_bneg[:], pbv[:, u : u + 1],
                        None, op0=OP.max, op1=OP.add,
                        accum_out=t_acc[:, u : u + 1],
                    )
                    nd += 1
            # tail pass (ACT relu over the tail block)
            nc.scalar.activation(
                t_trash_a[na % 2][:], t_bneg47[:], AF.Relu,
                bias=pbv[:, 16:17], scale=-1.0,
                accum_out=t_acc[:, 16:17],
            )

            nc.sync.dma_start(acc_out[:], t_acc[:])

    nc.finalize()
    return nc


def plan(outputs, labels, margin):
    X64 = np.asarray(outputs, dtype=np.float64)
    lab = np.asarray(labels).astype(np.int64).reshape(-1)
    margin = float(margin)
    assert X64.shape == (N, D) and lab.shape == (N,)

    Xb = X64.astype(BF)                      # round once
    Xw = Xb.astype(np.float64)               # exact value of the rounding
    r = (Xw * Xw).sum(1)

    nclass = max(C, int(lab.max()) + 1)
    m = np.bincount(lab, minlength=nclass)
    count = float(sum(int(mc) * (int(mc) - 1) * (N - int(mc)) for mc in m))

    rank = np.zeros(N, dtype=np.int64)
    cnt = {}
    for j in range(N):
        c = int(lab[j])
        rank[j] = cnt.get(c, 0)
        cnt[c] = cnt.get(c, 0) + 1
    members = {c: np.flatnonzero(lab == c)[np.argsort(rank[lab == c])]
               for c in range(nclass) if m[c]}

    # overflow (anchor, rank) pairs not covered by the 2*UMAIN main ranks
    overflow = []
    for c in range(nclass):
        if m[c] > 2 * UMAIN:
            mem = members[c]
            for i in mem:
                for rho in range(2 * UMAIN, int(m[c])):
                    if mem[rho] != i:
                        overflow.append((int(i), rho))
    assert len(overflow) <= NCORES * 128, (len(overflow), "tail overflow")

    key = margin
    if key not in _CACHE:
        _CACHE[key] = _build_program(margin)
    nc = _CACHE[key]

    r_hi = r.astype(BF)
    r_lo = (r - r_hi.astype(np.float64)).astype(BF)
    onehot = lab[None, :] == np.arange(nclass)[:C, None]  # [16, 512]

    def pos_dist(i, j):
        dd = Xw[i] - Xw[j]
        return np.sqrt((dd * dd).sum())

    def bias_col(anchor_ids, ranks):
        """bias per slot for (anchor, rank) pairs; anchor<0 -> pad."""
        nslot = len(anchor_ids)
        a = np.full(nslot, margin, dtype=np.float64)
        for p in range(nslot):
            i = anchor_ids[p]
            if i < 0:
                a[p] = -PADBIAS
                continue
            mem = members[int(lab[i])]
            rho = ranks[p]
            if rho < len(mem) and mem[rho] != i:
                a[p] = pos_dist(i, mem[rho]) + margin
        return a

    def aug_lhsT(anchor_ids):
        """[20, nslot] augment lhsT for given anchors (pad -> all-zero col)."""
        nslot = len(anchor_ids)
        L = np.zeros((20, nslot), dtype=BF)
        for p in range(nslot):
            i = anchor_ids[p]
            if i < 0:
                continue
            L[0, p] = 1.0
            L[1, p] = 1.0
            L[2 + int(lab[i]), p] = MASK
            L[18, p] = r_hi[i]
            L[19, p] = r_lo[i]
        return L

    # shared augment rhs [20, 512]
    RHS = np.zeros((20, 512), dtype=BF)
    RHS[0] = r_hi
    RHS[1] = r_lo
    RHS[2:18] = onehot.astype(np.float64)
    RHS[18] = 1.0
    RHS[19] = 1.0

    xgaT = (-2.0 * Xw).astype(BF).T  # [d, 512], exact pow-2 scale

    # split overflow across cores
    chunks = [overflow[c * 128:(c + 1) * 128] if False else None
              for c in range(NCORES)]
    per = (len(overflow) + NCORES - 1) // NCORES if overflow else 0
    chunks = [overflow[c * per:(c + 1) * per] for c in range(NCORES)]

    in_maps = []
    for c in range(NCORES):
        I = np.arange(c * APC, (c + 1) * APC)
        Idup = np.concatenate([I, I])
        pardup = np.concatenate([np.zeros(APC, np.int64),
                                 np.ones(APC, np.int64)])

        # tail slots for this core
        tail = chunks[c]
        t_anchor = np.full(128, -1, dtype=np.int64)
        t_rank = np.zeros(128, dtype=np.int64)
        for s, (i, rho) in enumerate(tail):
            t_anchor[s] = i
            t_rank[s] = rho

        PB = np.zeros((128, PBW), dtype=np.float32)
        main_ranks = np.empty((128, UMAIN), dtype=np.int64)
        for u in range(UMAIN):
            main_ranks[:, u] = 2 * u + pardup
            PB[:, u] = bias_col(Idup, main_ranks[:, u])
        PB[:, 16] = bias_col(t_anchor, t_rank)
        PB[:, 17] = -PADBIAS

        PA = np.empty((128, PAW), dtype=BF)
        PA[:, 0:128] = Xb[Idup].T
        PA[:, 128:640] = xgaT
        PA[:, 640:PAW] = PB.view(np.uint32).view(np.uint16).view(BF)

        PK2 = np.zeros((20, 768), dtype=BF)
        PK2[:, 0:128] = aug_lhsT(Idup)
        PK2[:, 128:640] = RHS
        PK2[:, 640:768] = aug_lhsT(t_anchor)

        PA47 = np.zeros((128, 128), dtype=BF)
        sel = t_anchor >= 0
        PA47[:, sel] = Xb[t_anchor[sel]].T

        in_maps.append({"pa": PA, "pk2": PK2, "pa47": PA47})

    return nc, in_maps, UMAIN, count


def reduce_results(results, umax, count):
    total = 0.0
    for c in range(NCORES):
        acc = results[c]["acc"].astype(np.float64)  # [128, PBW]
        rs = acc[:, 17].sum()
        for u in range(UMAIN):
            cs = acc[:, u].sum()
            total += cs if u in ACT_US else cs - rs
        total += acc[:, 16].sum()  # tail (ACT relu, direct)
    return np.float32(total / count)


def kernel(outputs, labels, margin):
    from concourse.bass_utils import run_bass_kernel_spmd

    nc, in_maps, umax, count = plan(outputs, labels, margin)
    res = run_bass_kernel_spmd(nc, in_maps, list(range(NCORES)))
    loss = reduce_results(res.results, umax, count)
    return (loss, 0.0, 0.0, 0.0)


# revision 4
# speedup vs baseline: 1.1199x; 1.1199x over previous
"""Trainium2 Bass kernel for batch-all triplet margin loss -- RAW Bass version.

Same math/host-prep as kernel_v2 (see its docstring), but the device program
is hand-scheduled raw Bass (no TileContext): manual semaphores, no Tile
prologue barrier (first DMA issues at ~50ns instead of ~670ns) and a
sem-only exit barrier instead of the full drain chain.

Queues:
  SP : dma pa -> dma pk2 -> dma pa47 -> (wait compute) dma acc out -> wait all
  PE : product (wait pa), augment (wait pk2) -> d2 | tail product+augment
  ACT: sqrt_main -> relu u14 -> sqrt47 -> relu u15 -> tail relu
  DVE: rowsum + u0..13 max/add passes (waits sqrt_main only)
"""

import numpy as np
import ml_dtypes

BF = ml_dtypes.bfloat16
N, D, C = 512, 128, 16
NCORES = 8
APC = N // NCORES
MASK = float(2.0**20)
PADBIAS = 1.0e9

UMAIN = 16
DVE_US = tuple(range(14))
ACT_US = (14, 15)
PBW = 18
PAW = 640 + 2 * PBW

_CACHE = {}


def _build_program(margin):
    import concourse.bacc as bacc
    from concourse import mybir

    fp32 = mybir.dt.float32
    bf16 = mybir.dt.bfloat16
    AF = mybir.ActivationFunctionType
    OP = mybir.AluOpType

    import concourse.bass as bass_mod
    _orig_barrier = bass_mod.Bass.all_engine_barrier
    bass_mod.Bass.all_engine_barrier = lambda self, **k: None
    nc = bacc.Bacc("TRN2", target_bir_lowering=False, debug=False)
    pa = nc.declare_dram_parameter("pa", [128, PAW], bf16, isOutput=False)
    pk2 = nc.declare_dram_parameter("pk2", [20, 768], bf16, isOutput=False)
    pa47 = nc.declare_dram_parameter("pa47", [128, 128], bf16, isOutput=False)
    acc_out = nc.declare_dram_parameter("acc", [128, PBW], fp32, isOutput=True)

    t_pa = nc.alloc_sbuf_tensor("t_pa", [128, PAW], bf16)
    t_pk2 = nc.alloc_sbuf_tensor("t_pk2", [20, 768], bf16)
    t_pa47 = nc.alloc_sbuf_tensor("t_pa47", [128, 128], bf16)
    t_bneg = nc.alloc_sbuf_tensor("t_bneg", [128, N], bf16)
    t_bneg47 = nc.alloc_sbuf_tensor("t_bneg47", [128, N], bf16)
    t_trd = nc.alloc_sbuf_tensor("t_trd", [128, 15 * N], bf16)
    t_tra = nc.alloc_sbuf_tensor("t_tra", [128, 3 * N], bf16)
    t_acc = nc.alloc_sbuf_tensor("t_acc", [128, PBW], fp32)
    p_d2 = nc.alloc_psum_tensor("p_d2", [128, N], fp32)
    p_d247 = nc.alloc_psum_tensor("p_d247", [128, N], fp32)

    csem = nc.alloc_semaphore("csem")
    wsem = nc.alloc_semaphore("wsem")
    t_warm = nc.alloc_sbuf_tensor("t_warm", [128, 256], bf16)
    nc.gpsimd.memset(t_warm[:], 1.0).then_inc(wsem, 1)
    t_mark = nc.alloc_sbuf_tensor("t_mark", [128, 1], fp32)
    nc.gpsimd.memset(t_mark[:], 0.0).then_inc(csem, 1)
    t_dact = nc.alloc_sbuf_tensor("t_dact", [128, 1], bf16)
    p_warm = nc.alloc_psum_tensor("p_warm", [128, 256], fp32)
    pasem = nc.alloc_semaphore("pasem")
    k2sem = nc.alloc_semaphore("k2sem")
    a47sem = nc.alloc_semaphore("a47sem")
    outsem = nc.alloc_semaphore("outsem")
    pesem = nc.alloc_semaphore("pesem")
    bsem = nc.alloc_semaphore("bsem")
    vsem = nc.alloc_semaphore("vsem")
    b47sem = nc.alloc_semaphore("b47sem")
    asem = nc.alloc_semaphore("asem")

    xia = t_pa[:, 0:128]
    xga = t_pa[:, 128:640]
    pbv = t_pa[:, 640:PAW].bitcast(fp32)

    with nc.Block(name="main", no_gpsimd_drain=True) as blk:

        @blk.sync
        def _(sync):
            sync.dma_start(t_pa[:], pa[:]).then_inc(pasem, 16)
            sync.dma_start(t_pk2[:], pk2[:]).then_inc(k2sem, 16)
            sync.dma_start(t_pa47[:], pa47[:]).then_inc(a47sem, 16)
            sync.wait_ge(vsem, 1)
            sync.wait_ge(asem, 1)
            sync.dma_start(acc_out[:], t_acc[:]).then_inc(outsem, 16)
            sync.wait_ge(outsem, 16)

        @blk.tensor
        def _(pe):
            pe.wait_ge(wsem, 1)
            for _ in range(6):
                pe.matmul(p_warm[:], t_warm[:, 0:128], t_warm[:, 0:256],
                          start=True, stop=True)
            pe.wait_ge(pasem, 16)
            pe.matmul(p_d2[:], xia, xga, start=True, stop=False)
            pe.wait_ge(k2sem, 16)
            pe.matmul(
                p_d2[:], t_pk2[0:20, 0:128], t_pk2[0:20, 128:640],
                start=False, stop=True,
            ).then_inc(pesem, 1)
            pe.wait_ge(a47sem, 16)
            pe.matmul(p_d247[:], t_pa47[:], xga, start=True, stop=False)
            pe.matmul(
                p_d247[:], t_pk2[0:20, 640:768], t_pk2[0:20, 128:640],
                start=False, stop=True,
            ).then_inc(pesem, 1)

        @blk.scalar
        def _(act):
            act.wait_ge(csem, 1)
            act.activation(t_dact[:], t_mark[:], AF.Sqrt)
            act.wait_ge(pesem, 1)
            act.activation(t_bneg[:], p_d2[:], AF.Sqrt).then_inc(bsem, 1)
            act.wait_ge(bsem, 1)
            act.activation(
                t_tra[:, 0:N], t_bneg[:], AF.Relu, bias=pbv[:, 14:15],
                scale=-1.0, accum_out=t_acc[:, 14:15],
            )
            act.wait_ge(pesem, 2)
            act.activation(t_bneg47[:], p_d247[:], AF.Sqrt).then_inc(b47sem, 1)
            act.activation(
                t_tra[:, N : 2 * N], t_bneg[:], AF.Relu, bias=pbv[:, 15:16],
                scale=-1.0, accum_out=t_acc[:, 15:16],
            )
            act.wait_ge(b47sem, 1)
            act.activation(
                t_tra[:, 2 * N : 3 * N], t_bneg47[:], AF.Relu,
                bias=pbv[:, 16:17], scale=-1.0, accum_out=t_acc[:, 16:17],
            ).then_inc(asem, 1)

        @blk.vector
        def _(dve):
            dve.wait_ge(bsem, 1)
            for k, u in enumerate(DVE_US):
                dve.tensor_scalar(
                    t_trd[:, k * N : (k + 1) * N], t_bneg[:],
                    pbv[:, u : u + 1], None,
                    op0=OP.max, op1=OP.add, accum_out=t_acc[:, u : u + 1],
                )
            dve.tensor_scalar(
                t_trd[:, 14 * N : 15 * N], t_bneg[:], pbv[:, 17:18], None,
                op0=OP.max, op1=OP.add, accum_out=t_acc[:, 17:18],
            ).then_inc(vsem, 1)

        pass
    bass_mod.Bass.all_engine_barrier = _orig_barrier
    nc.finalize()
    return nc


# ---- host prep: identical to kernel_v2 ----

def plan(outputs, labels, margin):
    X64 = np.asarray(outputs, dtype=np.float64)
    lab = np.asarray(labels).astype(np.int64).reshape(-1)
    margin = float(margin)
    assert X64.shape == (N, D) and lab.shape == (N,)

    Xb = X64.astype(BF)
    Xw = Xb.astype(np.float64)
    r = (Xw * Xw).sum(1)

    nclass = max(C, int(lab.max()) + 1)
    m = np.bincount(lab, minlength=nclass)
    count = float(sum(int(mc) * (int(mc) - 1) * (N - int(mc)) for mc in m))

    rank = np.zeros(N, dtype=np.int64)
    cnt = {}
    for j in range(N):
        c = int(lab[j])
        rank[j] = cnt.get(c, 0)
        cnt[c] = cnt.get(c, 0) + 1
    members = {c: np.flatnonzero(lab == c)[np.argsort(rank[lab == c])]
               for c in range(nclass) if m[c]}

    overflow = []
    for c in range(nclass):
        if m[c] > 2 * UMAIN:
            mem = members[c]
            for i in mem:
                for rho in range(2 * UMAIN, int(m[c])):
                    if mem[rho] != i:
                        overflow.append((int(i), rho))
    assert len(overflow) <= NCORES * 128, (len(overflow), "tail overflow")

    key = margin
    if key not in _CACHE:
        _CACHE[key] = _build_program(margin)
    nc = _CACHE[key]

    r_hi = r.astype(BF)
    r_lo = (r - r_hi.astype(np.float64)).astype(BF)
    onehot = lab[None, :] == np.arange(nclass)[:C, None]

    def pos_dist(i, j):
        dd = Xw[i] - Xw[j]
        return np.sqrt((dd * dd).sum())

    def bias_col(anchor_ids, ranks):
        nslot = len(anchor_ids)
        a = np.full(nslot, margin, dtype=np.float64)
        for p in range(nslot):
            i = anchor_ids[p]
            if i < 0:
                a[p] = -PADBIAS
                continue
            mem = members[int(lab[i])]
            rho = ranks[p]
            if rho < len(mem) and mem[rho] != i:
                a[p] = pos_dist(i, mem[rho]) + margin
        return a

    def aug_lhsT(anchor_ids):
        nslot = len(anchor_ids)
        L = np.zeros((20, nslot), dtype=BF)
        for p in range(nslot):
            i = anchor_ids[p]
            if i < 0:
                continue
            L[0, p] = 1.0
            L[1, p] = 1.0
            L[2 + int(lab[i]), p] = MASK
            L[18, p] = r_hi[i]
            L[19, p] = r_lo[i]
        return L

    RHS = np.zeros((20, 512), dtype=BF)
    RHS[0] = r_hi
    RHS[1] = r_lo
    RHS[2:18] = onehot.astype(np.float64)
    RHS[18] = 1.0
    RHS[19] = 1.0

    xgaT = (-2.0 * Xw).astype(BF).T

    per = (len(overflow) + NCORES - 1) // NCORES if overflow else 0
    chunks = [overflow[c * per:(c + 1) * per] for c in range(NCORES)]

    in_maps = []
    for c in range(NCORES):
        I = np.arange(c * APC, (c + 1) * APC)
        Idup = np.concatenate([I, I])
        pardup = np.concatenate([np.zeros(APC, np.int64),
                                 np.ones(APC, np.int64)])

        tail = chunks[c]
        t_anchor = np.full(128, -1, dtype=np.int64)
        t_rank = np.zeros(128, dtype=np.int64)
        for s, (i, rho) in enumerate(tail):
            t_anchor[s] = i
            t_rank[s] = rho

        PB = np.zeros((128, PBW), dtype=np.float32)
        for u in range(UMAIN):
            PB[:, u] = bias_col(Idup, 2 * u + pardup)
        PB[:, 16] = bias_col(t_anchor, t_rank)
        PB[:, 17] = -PADBIAS

        PA = np.empty((128, PAW), dtype=BF)
        PA[:, 0:128] = Xb[Idup].T
        PA[:, 128:640] = xgaT
        PA[:, 640:PAW] = PB.view(np.uint16).view(BF)

        PK2 = np.zeros((20, 768), dtype=BF)
        PK2[:, 0:128] = aug_lhsT(Idup)
        PK2[:, 128:640] = RHS
        PK2[:, 640:768] = aug_lhsT(t_anchor)

        PA47 = np.zeros((128, 128), dtype=BF)
        sel = t_anchor >= 0
        PA47[:, sel] = Xb[t_anchor[sel]].T

        in_maps.append({"pa": PA, "pk2": PK2, "pa47": PA47})

    return nc, in_maps, UMAIN, count


def reduce_results(results, umax, count):
    total = 0.0
    for c in range(NCORES):
        acc = results[c]["acc"].astype(np.float64)
        rs = acc[:, 17].sum()
        for u in range(UMAIN):
            cs = acc[:, u].sum()
            total += cs if u in ACT_US else cs - rs
        total += acc[:, 16].sum()
    return np.float32(total / count)


def kernel(outputs, labels, margin):
    from concourse.bass_utils import run_bass_kernel_spmd

    nc, in_maps, umax, count = plan(outputs, labels, margin)
    res = run_bass_kernel_spmd(nc, in_maps, list(range(NCORES)))
    loss = reduce_results(res.results, umax, count)
    return (loss, 0.0, 0.0, 0.0)


# revision 5
# speedup vs baseline: 1.1552x; 1.0316x over previous
"""Trainium2 Bass kernel for batch-all triplet margin loss -- RAW Bass version.

Same math/host-prep as kernel_v2 (see its docstring), but the device program
is hand-scheduled raw Bass (no TileContext): manual semaphores, no Tile
prologue barrier (first DMA issues at ~50ns instead of ~670ns) and a
sem-only exit barrier instead of the full drain chain.

Queues:
  SP : dma pa -> dma pk2 -> dma pa47 -> (wait compute) dma acc out -> wait all
  PE : product (wait pa), augment (wait pk2) -> d2 | tail product+augment
  ACT: sqrt_main -> relu u14 -> sqrt47 -> relu u15 -> tail relu
  DVE: rowsum + u0..13 max/add passes (waits sqrt_main only)
"""

import numpy as np
import ml_dtypes

BF = ml_dtypes.bfloat16
N, D, C = 512, 128, 16
NCORES = 8
APC = N // NCORES
MASK = float(2.0**20)
PADBIAS = 1.0e9

UMAIN = 16
DVE_US = tuple(range(14))
ACT_US = (14, 15)
PBW = 18
PAW = 640 + 2 * PBW

_CACHE = {}


def _build_program(margin):
    import concourse.bacc as bacc
    from concourse import mybir

    fp32 = mybir.dt.float32
    bf16 = mybir.dt.bfloat16
    AF = mybir.ActivationFunctionType
    OP = mybir.AluOpType

    import concourse.bass as bass_mod
    _orig_barrier = bass_mod.Bass.all_engine_barrier
    bass_mod.Bass.all_engine_barrier = lambda self, **k: None
    nc = bacc.Bacc("TRN2", target_bir_lowering=False, debug=False)
    pa = nc.declare_dram_parameter("pa", [128, PAW], bf16, isOutput=False)
    pk2 = nc.declare_dram_parameter("pk2", [20, 768], bf16, isOutput=False)
    pa47 = nc.declare_dram_parameter("pa47", [128, 128], bf16, isOutput=False)
    acc_out = nc.declare_dram_parameter("acc", [128, PBW], fp32, isOutput=True)

    t_pa = nc.alloc_sbuf_tensor("t_pa", [128, PAW], bf16)
    t_pk2 = nc.alloc_sbuf_tensor("t_pk2", [20, 768], bf16)
    t_pa47 = nc.alloc_sbuf_tensor("t_pa47", [128, 128], bf16)
    t_bneg = nc.alloc_sbuf_tensor("t_bneg", [128, N], bf16)
    t_bneg47 = nc.alloc_sbuf_tensor("t_bneg47", [128, N], bf16)
    t_trd = nc.alloc_sbuf_tensor("t_trd", [128, 15 * N], bf16)
    t_tra = nc.alloc_sbuf_tensor("t_tra", [128, 3 * N], bf16)
    t_acc = nc.alloc_sbuf_tensor("t_acc", [128, PBW], fp32)
    p_d2 = nc.alloc_psum_tensor("p_d2", [128, N], fp32)
    p_d247 = nc.alloc_psum_tensor("p_d247", [128, N], fp32)

    csem = nc.alloc_semaphore("csem")
    wsem = nc.alloc_semaphore("wsem")
    t_warm = nc.alloc_sbuf_tensor("t_warm", [128, 256], bf16)
    nc.gpsimd.memset(t_warm[:], 1.0).then_inc(wsem, 1)
    t_mark = nc.alloc_sbuf_tensor("t_mark", [128, 1], fp32)
    nc.gpsimd.memset(t_mark[:], 0.0).then_inc(csem, 1)
    t_dact = nc.alloc_sbuf_tensor("t_dact", [128, 1], bf16)
    p_warm = nc.alloc_psum_tensor("p_warm", [128, 256], fp32)
    pasem = nc.alloc_semaphore("pasem")
    k2sem = nc.alloc_semaphore("k2sem")
    a47sem = nc.alloc_semaphore("a47sem")
    outsem = nc.alloc_semaphore("outsem")
    pesem = nc.alloc_semaphore("pesem")
    bsem = nc.alloc_semaphore("bsem")
    vsem = nc.alloc_semaphore("vsem")
    b47sem = nc.alloc_semaphore("b47sem")
    asem = nc.alloc_semaphore("asem")

    xia = t_pa[:, 0:128]
    xga = t_pa[:, 128:640]
    pbv = t_pa[:, 640:PAW].bitcast(fp32)

    with nc.Block(name="main", no_gpsimd_drain=True) as blk:

        @blk.sync
        def _(sync):
            sync.dma_start(t_pa[:], pa[:]).then_inc(pasem, 16)
            sync.dma_start(t_pk2[:], pk2[:]).then_inc(k2sem, 16)
            sync.dma_start(t_pa47[:], pa47[:]).then_inc(a47sem, 16)
            sync.wait_ge(vsem, 1)
            sync.wait_ge(asem, 1)
            sync.dma_start(acc_out[:], t_acc[:]).then_inc(outsem, 16)
            sync.wait_ge(outsem, 16)

        @blk.tensor
        def _(pe):
            pe.wait_ge(wsem, 1)
            for _ in range(6):
                pe.matmul(p_warm[:], t_warm[:, 0:128], t_warm[:, 0:256],
                          start=True, stop=True)
            pe.wait_ge(pasem, 16)
            pe.matmul(p_d2[:], xia, xga, start=True, stop=False)
            pe.wait_ge(k2sem, 16)
            pe.matmul(
                p_d2[:], t_pk2[0:20, 0:128], t_pk2[0:20, 128:640],
                start=False, stop=True,
            ).then_inc(pesem, 1)
            pe.wait_ge(a47sem, 16)
            pe.matmul(p_d247[:], t_pa47[:], xga, start=True, stop=False)
            pe.matmul(
                p_d247[:], t_pk2[0:20, 640:768], t_pk2[0:20, 128:640],
                start=False, stop=True,
            ).then_inc(pesem, 1)

        @blk.scalar
        def _(act):
            act.wait_ge(csem, 1)
            act.activation(t_dact[:], t_mark[:], AF.Sqrt)
            act.wait_ge(pesem, 1)
            act.activation(t_bneg[:], p_d2[:], AF.Sqrt).then_inc(bsem, 1)
            act.activation(
                t_tra[:, 0:N], t_bneg[:], AF.Relu, bias=pbv[:, 14:15],
                scale=-1.0, accum_out=t_acc[:, 14:15],
            )
            act.wait_ge(pesem, 2)
            act.activation(t_bneg47[:], p_d247[:], AF.Sqrt).then_inc(b47sem, 1)
            act.activation(
                t_tra[:, N : 2 * N], t_bneg[:], AF.Relu, bias=pbv[:, 15:16],
                scale=-1.0, accum_out=t_acc[:, 15:16],
            )
            act.wait_ge(b47sem, 1)
            act.activation(
                t_tra[:, 2 * N : 3 * N], t_bneg47[:], AF.Relu,
                bias=pbv[:, 16:17], scale=-1.0, accum_out=t_acc[:, 16:17],
            ).then_inc(asem, 1)

        @blk.vector
        def _(dve):
            dve.wait_ge(bsem, 1)
            for k, u in enumerate(DVE_US):
                dve.tensor_scalar(
                    t_trd[:, k * N : (k + 1) * N], t_bneg[:],
                    pbv[:, u : u + 1], None,
                    op0=OP.max, op1=OP.add, accum_out=t_acc[:, u : u + 1],
                )
            dve.tensor_scalar(
                t_trd[:, 14 * N : 15 * N], t_bneg[:], pbv[:, 17:18], None,
                op0=OP.max, op1=OP.add, accum_out=t_acc[:, 17:18],
            ).then_inc(vsem, 1)

        pass
    bass_mod.Bass.all_engine_barrier = _orig_barrier
    nc.finalize()
    return nc


# ---- host prep: identical to kernel_v2 ----

def plan(outputs, labels, margin):
    X64 = np.asarray(outputs, dtype=np.float64)
    lab = np.asarray(labels).astype(np.int64).reshape(-1)
    margin = float(margin)
    assert X64.shape == (N, D) and lab.shape == (N,)

    Xb = X64.astype(BF)
    Xw = Xb.astype(np.float64)
    r = (Xw * Xw).sum(1)

    nclass = max(C, int(lab.max()) + 1)
    m = np.bincount(lab, minlength=nclass)
    count = float(sum(int(mc) * (int(mc) - 1) * (N - int(mc)) for mc in m))

    rank = np.zeros(N, dtype=np.int64)
    cnt = {}
    for j in range(N):
        c = int(lab[j])
        rank[j] = cnt.get(c, 0)
        cnt[c] = cnt.get(c, 0) + 1
    members = {c: np.flatnonzero(lab == c)[np.argsort(rank[lab == c])]
               for c in range(nclass) if m[c]}

    overflow = []
    for c in range(nclass):
        if m[c] > 2 * UMAIN:
            mem = members[c]
            for i in mem:
                for rho in range(2 * UMAIN, int(m[c])):
                    if mem[rho] != i:
                        overflow.append((int(i), rho))
    assert len(overflow) <= NCORES * 128, (len(overflow), "tail overflow")

    key = margin
    if key not in _CACHE:
        _CACHE[key] = _build_program(margin)
    nc = _CACHE[key]

    r_hi = r.astype(BF)
    r_lo = (r - r_hi.astype(np.float64)).astype(BF)
    onehot = lab[None, :] == np.arange(nclass)[:C, None]

    def pos_dist(i, j):
        dd = Xw[i] - Xw[j]
        return np.sqrt((dd * dd).sum())

    def bias_col(anchor_ids, ranks):
        nslot = len(anchor_ids)
        a = np.full(nslot, margin, dtype=np.float64)
        for p in range(nslot):
            i = anchor_ids[p]
            if i < 0:
                a[p] = -PADBIAS
                continue
            mem = members[int(lab[i])]
            rho = ranks[p]
            if rho < len(mem) and mem[rho] != i:
                a[p] = pos_dist(i, mem[rho]) + margin
        return a

    def aug_lhsT(anchor_ids):
        nslot = len(anchor_ids)
        L = np.zeros((20, nslot), dtype=BF)
        for p in range(nslot):
            i = anchor_ids[p]
            if i < 0:
                continue
            L[0, p] = 1.0
            L[1, p] = 1.0
            L[2 + int(lab[i]), p] = MASK
            L[18, p] = r_hi[i]
            L[19, p] = r_lo[i]
        return L

    RHS = np.zeros((20, 512), dtype=BF)
    RHS[0] = r_hi
    RHS[1] = r_lo
    RHS[2:18] = onehot.astype(np.float64)
    RHS[18] = 1.0
    RHS[19] = 1.0

    xgaT = (-2.0 * Xw).astype(BF).T

    per = (len(overflow) + NCORES - 1) // NCORES if overflow else 0
    chunks = [overflow[c * per:(c + 1) * per] for c in range(NCORES)]

    in_maps = []
    for c in range(NCORES):
        I = np.arange(c * APC, (c + 1) * APC)
        Idup = np.concatenate([I, I])
        pardup = np.concatenate([np.zeros(APC, np.int64),
                                 np.ones(APC, np.int64)])

        tail = chunks[c]
        t_anchor = np.full(128, -1, dtype=np.int64)
        t_rank = np.zeros(128, dtype=np.int64)
        for s, (i, rho) in enumerate(tail):
            t_anchor[s] = i
            t_rank[s] = rho

        PB = np.zeros((128, PBW), dtype=np.float32)
        for u in range(UMAIN):
            PB[:, u] = bias_col(Idup, 2 * u + pardup)
        PB[:, 16] = bias_col(t_anchor, t_rank)
        PB[:, 17] = -PADBIAS

        PA = np.empty((128, PAW), dtype=BF)
        PA[:, 0:128] = Xb[Idup].T
        PA[:, 128:640] = xgaT
        PA[:, 640:PAW] = PB.view(np.uint16).view(BF)

        PK2 = np.zeros((20, 768), dtype=BF)
        PK2[:, 0:128] = aug_lhsT(Idup)
        PK2[:, 128:640] = RHS
        PK2[:, 640:768] = aug_lhsT(t_anchor)

        PA47 = np.zeros((128, 128), dtype=BF)
        sel = t_anchor >= 0
        PA47[:, sel] = Xb[t_anchor[sel]].T

        in_maps.append({"pa": PA, "pk2": PK2, "pa47": PA47})

    return nc, in_maps, UMAIN, count


def reduce_results(results, umax, count):
    total = 0.0
    for c in range(NCORES):
        acc = results[c]["acc"].astype(np.float64)
        rs = acc[:, 17].sum()
        for u in range(UMAIN):
            cs = acc[:, u].sum()
            total += cs if u in ACT_US else cs - rs
        total += acc[:, 16].sum()
    return np.float32(total / count)


def kernel(outputs, labels, margin):
    from concourse.bass_utils import run_bass_kernel_spmd

    nc, in_maps, umax, count = plan(outputs, labels, margin)
    res = run_bass_kernel_spmd(nc, in_maps, list(range(NCORES)))
    loss = reduce_results(res.results, umax, count)
    return (loss, 0.0, 0.0, 0.0)


# revision 6
# speedup vs baseline: 1.1556x; 1.0003x over previous
"""Trainium2 Bass kernel for batch-all triplet margin loss -- RAW Bass version.

Same math/host-prep as kernel_v2 (see its docstring), but the device program
is hand-scheduled raw Bass (no TileContext): manual semaphores, no Tile
prologue barrier (first DMA issues at ~50ns instead of ~670ns) and a
sem-only exit barrier instead of the full drain chain.

Queues:
  SP : dma pa -> dma pk2 -> dma pa47 -> (wait compute) dma acc out -> wait all
  PE : product (wait pa), augment (wait pk2) -> d2 | tail product+augment
  ACT: sqrt_main -> relu u14 -> sqrt47 -> relu u15 -> tail relu
  DVE: rowsum + u0..13 max/add passes (waits sqrt_main only)
"""

import numpy as np
import ml_dtypes

BF = ml_dtypes.bfloat16
N, D, C = 512, 128, 16
NCORES = 8
APC = N // NCORES
MASK = float(2.0**20)
PADBIAS = 1.0e9

UMAIN = 16
DVE_US = tuple(range(14))
ACT_US = (14, 15)
PBW = 18
ACCW = 22
PAW = 640 + 2 * PBW

_CACHE = {}


def _build_program(margin):
    import concourse.bacc as bacc
    from concourse import mybir

    fp32 = mybir.dt.float32
    bf16 = mybir.dt.bfloat16
    AF = mybir.ActivationFunctionType
    OP = mybir.AluOpType

    import concourse.bass as bass_mod
    _orig_barrier = bass_mod.Bass.all_engine_barrier
    bass_mod.Bass.all_engine_barrier = lambda self, **k: None
    nc = bacc.Bacc("TRN2", target_bir_lowering=False, debug=False)
    pa = nc.declare_dram_parameter("pa", [128, PAW], bf16, isOutput=False)
    pk2 = nc.declare_dram_parameter("pk2", [20, 768], bf16, isOutput=False)
    pa47 = nc.declare_dram_parameter("pa47", [128, 128], bf16, isOutput=False)
    acc_out = nc.declare_dram_parameter("acc", [128, ACCW], fp32, isOutput=True)

    t_pa = nc.alloc_sbuf_tensor("t_pa", [128, PAW], bf16)
    t_pk2 = nc.alloc_sbuf_tensor("t_pk2", [20, 768], bf16)
    t_pa47 = nc.alloc_sbuf_tensor("t_pa47", [128, 128], bf16)
    t_bneg = nc.alloc_sbuf_tensor("t_bneg", [128, N], bf16)
    t_bneg47 = nc.alloc_sbuf_tensor("t_bneg47", [128, N], bf16)
    t_trd = nc.alloc_sbuf_tensor("t_trd", [128, 15 * N], bf16)
    t_tra = nc.alloc_sbuf_tensor("t_tra", [128, 3 * N], bf16)
    t_acc = nc.alloc_sbuf_tensor("t_acc", [128, ACCW], fp32)
    p_d2l = nc.alloc_psum_tensor("p_d2l", [128, 256], fp32)
    p_d2h = nc.alloc_psum_tensor("p_d2h", [128, 256], fp32)
    p_d247 = nc.alloc_psum_tensor("p_d247", [128, N], fp32)

    csem = nc.alloc_semaphore("csem")
    wsem = nc.alloc_semaphore("wsem")
    t_warm = nc.alloc_sbuf_tensor("t_warm", [128, 256], bf16)
    nc.gpsimd.memset(t_warm[:], 1.0).then_inc(wsem, 1)
    t_mark = nc.alloc_sbuf_tensor("t_mark", [128, 1], fp32)
    nc.gpsimd.memset(t_mark[:], 0.0).then_inc(csem, 1)
    t_dact = nc.alloc_sbuf_tensor("t_dact", [128, 1], bf16)
    p_warm = nc.alloc_psum_tensor("p_warm", [128, 256], fp32)
    pasem = nc.alloc_semaphore("pasem")
    k2sem = nc.alloc_semaphore("k2sem")
    a47sem = nc.alloc_semaphore("a47sem")
    outsem = nc.alloc_semaphore("outsem")
    pesem = nc.alloc_semaphore("pesem")
    bsem = nc.alloc_semaphore("bsem")
    vsem = nc.alloc_semaphore("vsem")
    b47sem = nc.alloc_semaphore("b47sem")
    asem = nc.alloc_semaphore("asem")

    xia = t_pa[:, 0:128]
    xga = t_pa[:, 128:640]
    pbv = t_pa[:, 640:PAW].bitcast(fp32)

    with nc.Block(name="main", no_gpsimd_drain=True) as blk:

        @blk.sync
        def _(sync):
            sync.dma_start(t_pa[:], pa[:]).then_inc(pasem, 16)
            sync.dma_start(t_pk2[:], pk2[:]).then_inc(k2sem, 16)
            sync.dma_start(t_pa47[:], pa47[:]).then_inc(a47sem, 16)
            sync.wait_ge(vsem, 1)
            sync.wait_ge(asem, 1)
            sync.dma_start(acc_out[:], t_acc[:]).then_inc(outsem, 16)
            sync.wait_ge(outsem, 16)

        @blk.tensor
        def _(pe):
            pe.wait_ge(wsem, 1)
            for _ in range(6):
                pe.matmul(p_warm[:], t_warm[:, 0:128], t_warm[:, 0:256],
                          start=True, stop=True)
            pe.wait_ge(pasem, 16)
            pe.matmul(p_d2l[:], xia, xga[:, 0:256],
                      start=True, stop=False)
            pe.wait_ge(k2sem, 16)
            pe.matmul(
                p_d2l[:], t_pk2[0:20, 0:128], t_pk2[0:20, 128:384],
                start=False, stop=True,
            ).then_inc(pesem, 1)
            pe.matmul(p_d2h[:], xia, xga[:, 256:512],
                      start=True, stop=False)
            pe.matmul(
                p_d2h[:], t_pk2[0:20, 0:128], t_pk2[0:20, 384:640],
                start=False, stop=True,
            ).then_inc(pesem, 1)
            pe.wait_ge(a47sem, 16)
            pe.matmul(p_d247[:], t_pa47[:], xga, start=True, stop=False)
            pe.matmul(
                p_d247[:], t_pk2[0:20, 640:768], t_pk2[0:20, 128:640],
                start=False, stop=True,
            ).then_inc(pesem, 1)

        @blk.scalar
        def _(act):
            act.wait_ge(csem, 1)
            act.activation(t_dact[:], t_mark[:], AF.Sqrt)
            act.wait_ge(pesem, 1)
            act.activation(t_bneg[:, 0:256], p_d2l[:], AF.Sqrt).then_inc(bsem, 1)
            act.wait_ge(pesem, 2)
            act.activation(
                t_bneg[:, 256:512], p_d2h[:], AF.Sqrt
            ).then_inc(bsem, 1)
            act.activation(
                t_tra[:, 0:N], t_bneg[:], AF.Relu, bias=pbv[:, 14:15],
                scale=-1.0, accum_out=t_acc[:, 14:15],
            )
            act.wait_ge(pesem, 3)
            act.activation(t_bneg47[:], p_d247[:], AF.Sqrt).then_inc(b47sem, 1)
            act.activation(
                t_tra[:, N : 2 * N], t_bneg[:], AF.Relu, bias=pbv[:, 15:16],
                scale=-1.0, accum_out=t_acc[:, 15:16],
            )
            act.wait_ge(b47sem, 1)
            act.activation(
                t_tra[:, 2 * N : 3 * N], t_bneg47[:], AF.Relu,
                bias=pbv[:, 16:17], scale=-1.0, accum_out=t_acc[:, 16:17],
            ).then_inc(asem, 1)

        @blk.vector
        def _(dve):
            H = 256
            dve.wait_ge(bsem, 1)
            for j in range(3):          # early lo-half passes (u0..u2)
                dve.tensor_scalar(
                    t_trd[:, j * N : j * N + H], t_bneg[:, 0:H],
                    pbv[:, j : j + 1], None,
                    op0=OP.max, op1=OP.add,
                    accum_out=t_acc[:, 18 + j : 19 + j],
                )
            dve.wait_ge(bsem, 2)
            for j in range(3):          # hi halves of u0..u2
                dve.tensor_scalar(
                    t_trd[:, j * N + H : (j + 1) * N], t_bneg[:, H:N],
                    pbv[:, j : j + 1], None,
                    op0=OP.max, op1=OP.add, accum_out=t_acc[:, j : j + 1],
                )
            for k, u in enumerate(DVE_US[3:]):  # full passes u3..u13
                dve.tensor_scalar(
                    t_trd[:, (3 + k) * N : (4 + k) * N], t_bneg[:],
                    pbv[:, u : u + 1], None,
                    op0=OP.max, op1=OP.add, accum_out=t_acc[:, u : u + 1],
                )
            dve.tensor_scalar(          # rowsum lo -> col 17
                t_trd[:, 14 * N : 14 * N + H], t_bneg[:, 0:H],
                pbv[:, 17:18], None,
                op0=OP.max, op1=OP.add, accum_out=t_acc[:, 17:18],
            )
            dve.tensor_scalar(          # rowsum hi -> col 21
                t_trd[:, 14 * N + H : 15 * N], t_bneg[:, H:N],
                pbv[:, 17:18], None,
                op0=OP.max, op1=OP.add, accum_out=t_acc[:, 21:22],
            ).then_inc(vsem, 1)

        pass
    bass_mod.Bass.all_engine_barrier = _orig_barrier
    nc.finalize()
    return nc


# ---- host prep: identical to kernel_v2 ----

def plan(outputs, labels, margin):
    X64 = np.asarray(outputs, dtype=np.float64)
    lab = np.asarray(labels).astype(np.int64).reshape(-1)
    margin = float(margin)
    assert X64.shape == (N, D) and lab.shape == (N,)

    Xb = X64.astype(BF)
    Xw = Xb.astype(np.float64)
    r = (Xw * Xw).sum(1)

    nclass = max(C, int(lab.max()) + 1)
    m = np.bincount(lab, minlength=nclass)
    count = float(sum(int(mc) * (int(mc) - 1) * (N - int(mc)) for mc in m))

    rank = np.zeros(N, dtype=np.int64)
    cnt = {}
    for j in range(N):
        c = int(lab[j])
        rank[j] = cnt.get(c, 0)
        cnt[c] = cnt.get(c, 0) + 1
    members = {c: np.flatnonzero(lab == c)[np.argsort(rank[lab == c])]
               for c in range(nclass) if m[c]}

    overflow = []
    for c in range(nclass):
        if m[c] > 2 * UMAIN:
            mem = members[c]
            for i in mem:
                for rho in range(2 * UMAIN, int(m[c])):
                    if mem[rho] != i:
                        overflow.append((int(i), rho))
    assert len(overflow) <= NCORES * 128, (len(overflow), "tail overflow")

    key = margin
    if key not in _CACHE:
        _CACHE[key] = _build_program(margin)
    nc = _CACHE[key]

    r_hi = r.astype(BF)
    r_lo = (r - r_hi.astype(np.float64)).astype(BF)
    onehot = lab[None, :] == np.arange(nclass)[:C, None]

    def pos_dist(i, j):
        dd = Xw[i] - Xw[j]
        return np.sqrt((dd * dd).sum())

    def bias_col(anchor_ids, ranks):
        nslot = len(anchor_ids)
        a = np.full(nslot, margin, dtype=np.float64)
        for p in range(nslot):
            i = anchor_ids[p]
            if i < 0:
                a[p] = -PADBIAS
                continue
            mem = members[int(lab[i])]
            rho = ranks[p]
            if rho < len(mem) and mem[rho] != i:
                a[p] = pos_dist(i, mem[rho]) + margin
        return a

    def aug_lhsT(anchor_ids):
        nslot = len(anchor_ids)
        L = np.zeros((20, nslot), dtype=BF)
        for p in range(nslot):
            i = anchor_ids[p]
            if i < 0:
                continue
            L[0, p] = 1.0
            L[1, p] = 1.0
            L[2 + int(lab[i]), p] = MASK
            L[18, p] = r_hi[i]
            L[19, p] = r_lo[i]
        return L

    RHS = np.zeros((20, 512), dtype=BF)
    RHS[0] = r_hi
    RHS[1] = r_lo
    RHS[2:18] = onehot.astype(np.float64)
    RHS[18] = 1.0
    RHS[19] = 1.0

    xgaT = (-2.0 * Xw).astype(BF).T

    per = (len(overflow) + NCORES - 1) // NCORES if overflow else 0
    chunks = [overflow[c * per:(c + 1) * per] for c in range(NCORES)]

    in_maps = []
    for c in range(NCORES):
        I = np.arange(c * APC, (c + 1) * APC)
        Idup = np.concatenate([I, I])
        pardup = np.concatenate([np.zeros(APC, np.int64),
                                 np.ones(APC, np.int64)])

        tail = chunks[c]
        t_anchor = np.full(128, -1, dtype=np.int64)
        t_rank = np.zeros(128, dtype=np.int64)
        for s, (i, rho) in enumerate(tail):
            t_anchor[s] = i
            t_rank[s] = rho

        PB = np.zeros((128, PBW), dtype=np.float32)
        for u in range(UMAIN):
            PB[:, u] = bias_col(Idup, 2 * u + pardup)
        PB[:, 16] = bias_col(t_anchor, t_rank)
        PB[:, 17] = -PADBIAS

        PA = np.empty((128, PAW), dtype=BF)
        PA[:, 0:128] = Xb[Idup].T
        PA[:, 128:640] = xgaT
        PA[:, 640:PAW] = PB.view(np.uint16).view(BF)

        PK2 = np.zeros((20, 768), dtype=BF)
        PK2[:, 0:128] = aug_lhsT(Idup)
        PK2[:, 128:640] = RHS
        PK2[:, 640:768] = aug_lhsT(t_anchor)

        PA47 = np.zeros((128, 128), dtype=BF)
        sel = t_anchor >= 0
        PA47[:, sel] = Xb[t_anchor[sel]].T

        in_maps.append({"pa": PA, "pk2": PK2, "pa47": PA47})

    return nc, in_maps, UMAIN, count


def reduce_results(results, umax, count):
    total = 0.0
    for c in range(NCORES):
        acc = results[c]["acc"].astype(np.float64)
        rs = acc[:, 17].sum() + acc[:, 21].sum()
        for u in range(UMAIN):
            cs = acc[:, u].sum()
            if u < 3:
                cs += acc[:, 18 + u].sum()
            total += cs if u in ACT_US else cs - rs
        total += acc[:, 16].sum()
    return np.float32(total / count)


def kernel(outputs, labels, margin):
    from concourse.bass_utils import run_bass_kernel_spmd

    nc, in_maps, umax, count = plan(outputs, labels, margin)
    res = run_bass_kernel_spmd(nc, in_maps, list(range(NCORES)))
    loss = reduce_results(res.results, umax, count)
    return (loss, 0.0, 0.0, 0.0)
